# revision 42
# baseline (speedup 1.0000x reference)
"""Trainium2 Bass kernel for nn_CrossCorrelLoss.

Math: for input X of shape (B=32, T=1024, D=321) the reference computes
  mu, sd over all (B,T) per feature; Xs = (X-mu)/sd;
  ccf = mean_b [Xs_b^T Xs_b / T]  (lower-triangle entries);
  loss = sum |ccf_fake - ccf_real| / 10.
Since mean_b of the per-batch Grams equals the flat Gram over all N=B*T rows,
everything reduces to the raw moments S1 = colsum(X) and S2 = X^T X:
  G = (S2/N - mu mu^T) / (sd sd^T),  mu = S1/N,
  var = (diag(S2) - N mu^2)/(N-1).
We append a ones column to X on the host (input marshalling), so a single
augmented Gram S2a = [X|1]^T [X|1] of shape (322, 322) carries S2, S1 and N.

Precision: inputs are cast to fp8 e4m3 on the host. Products of two e4m3
values are exact in the PE's internal format and accumulate in fp32 PSUM
over 32768 rows, so per-element quantization noise averages out across the
Gram reduction: measured end-to-end relative error is ~1e-5 against the fp32
reference, far inside the 2e-2 gate. fp8 also halves PE time via DoubleRow
perf mode and quarters HBM traffic vs fp32 — this kernel is HBM-bound
(2.64MB of input per core ~= 7.3us at the modeled 360 B/ns DMA bus).

Device schedule (data-parallel over batch, 4 batches = 4096 rows per core;
raw Bass with hand-managed semaphores — no Tile framework, whose region-exit
barrier would add ~1us between the final accumulation and the store):
  * One big SBUF tile per tensor holds all 32 row-chunks of 128 rows. The
    DoubleRow dual-stride restriction (16B-aligned byte stride between the
    two contracted k-tiles) pins the pairing to SBUF positions (s, s+16)
    [stride 16*322B = 5152B]. Each input DMA loads SBUF position sets
    {a..a+k-1} u {16+a..16+a+k-1} from one contiguous DRAM range, so every
    DMA completes k whole pairs and the PE can start on them immediately.
    All input DMAs sit on the SP HWDGE queue: deterministic global config
    order, and the first group is >=3 pairs so its transfer outlasts the
    ~650ns per-DMA config cadence and the DMA bus never idles. The last
    group is 2 pairs (the minimum at full DMA efficiency) so the final
    input DMA gates only a short PE batch.
  * Per pair, three upper-triangle block matmuls accumulate into PSUM:
      b0: rows 0:128   x cols 0:322   -> psA[:, 0:322]
      b1: rows 128:256 x cols 128:322 -> psB[:, 0:194]
      b2: rows 256:322 x cols 256:322 -> psB[0:66, 194:260]
    psB packs b1 and b2 as ONE accumulation group: only b1's first matmul
    carries `start` (which zeroes the whole PSUM zero region, covering
    b2's bytes) and only b2's last carries `stop`. Two start groups in one
    zero region corrupt each other (the second `start` re-arms
    pending-zero over the first group's partial sums).
  * Drain: psA -> Act engine copy, psB -> DVE copy (GPSIMD cannot access
    PSUM), converting to fp16 staging; then a single HWDGE store per
    tensor from the SP queue. The xf chain hides entirely under the xr
    input stream; only xr's drain+store chain is exposed at the tail.
    (A prepared SWDGE kv_writeback store would model a much shorter tail,
    but the GPSIMD SWDGE path hard-faults the NeuronCore in this runtime —
    NRT_EXEC_UNIT_UNRECOVERABLE even for a plain gpsimd dma_start. DMA
    directly from PSUM is also rejected by the stack.)
  * The framework preamble's const-AP memsets and the all-engine barrier
    fencing them are stripped: nothing here reads the const APs, and each
    engine's register preamble is ordered by its own queue. This moves the
    first input byte from ~1.9us to ~1.3us.
Host: sum the 8 per-core partial Grams (the all-reduce over B), symmetrize,
then the tiny (322x322) postprocessing in float64.
"""

import numpy as np
import ml_dtypes

import concourse.bacc as bacc
import concourse.bass as bass
import concourse.mybir as mybir
from concourse import bass_utils

N_CORES = 8
B, T, D = 32, 1024, 321
DA = D + 1  # 322: features + ones column
ROWS_PER_CORE = (B // N_CORES) * T  # 4096
P = 128  # partitions / contraction tile
N_CHUNKS = ROWS_PER_CORE // P  # 32
N_PAIRS = N_CHUNKS // 2  # 16 DoubleRow pairs, pairing (s, s+16)

IN_DT = mybir.dt.float8e4
IN_NP = ml_dtypes.float8_e4m3
OUT_DT = mybir.dt.float32  # PSUM accumulator dtype
# Partial Grams travel back as fp16 (3 extra mantissa bits vs bf16; Gram
# entries are bounded by ~N=4096 so fp16's 65504 range is safe): the host
# sums them in float64.
ST_DT = mybir.dt.float16

# Upper-triangle blocks of the Gram as matmuls:
# (row_lo, row_hi, col_lo, col_hi, psum_idx, psum_col_off, start, stop,
# pe_sem_idx) — start/stop only on the marked pairs (one accumulation
# group per PSUM tensor); pe_sem_idx names the completion sem the last
# pair's matmul bumps. (Splitting b1 across psA/psB to balance the two
# drain copies was tried: the 4th matmul per pair costs more on the PE
# dispatch path than the rebalance saves.)
MM_BLOCKS = [
    (0, 128, 0, DA, 0, 0, True, True, 0),  # b0
    (128, 256, 128, DA, 1, 0, True, False, None),  # b1
    (256, DA, 256, DA, 1, DA - 128, False, True, 1),  # b2
]
PS_W = [DA, (DA - 128) + (DA - 256)]  # 322, 260
OUT_W = PS_W[0] + PS_W[1]  # 582

# Input DMA sizes in PAIRS per tensor (sum = 16 each).
PAIR_GROUPS = ([3, 5, 8], [8, 4, 2, 2])

_NC_CACHE = {}


def _build_program(pair_groups=PAIR_GROUPS):
    # Bacc (not raw Bass class): its compile() pass legalizes multi-wait
    # instructions, which walrus otherwise rejects.
    nc = bacc.Bacc(trn_type="TRN2", target_bir_lowering=False, debug=False)

    # Strip the framework preamble's const-AP memsets and the all-engine
    # barrier that fences them (see module docstring).
    entry = list(nc.main_func.blocks)[0]
    il = entry.instructions
    for idx in range(len(il) - 1, -1, -1):
        i = il[idx]
        drop = False
        if isinstance(i, mybir.InstMemset):
            try:
                drop = str(i.outs[0].memref).startswith("const-")
            except Exception:
                drop = False
        elif isinstance(i, (mybir.InstDrain, mybir.InstEventSemaphore)):
            drop = True
        if drop:
            del il[idx]

    bases = ("xf", "xr")
    ins = {}
    outs = {}
    for key in bases:
        ins[key] = nc.dram_tensor(
            key, [ROWS_PER_CORE, DA], IN_DT, kind="ExternalInput"
        ).ap()
        outs[key] = nc.dram_tensor(
            "g" + key[1], [P, OUT_W], ST_DT, kind="ExternalOutput"
        ).ap()

    st = {b: nc.alloc_sbuf_tensor(f"st_{b}", [P, OUT_W], ST_DT) for b in bases}
    for b in bases:
        # rows 66:128 of the b2 staging region are never written by the
        # drain copies; zero the staging buffers up front (program start,
        # ~9us before any store reads them) so the store never reads
        # uninitialized SBUF.
        nc.gpsimd.memset(st[b].ap(), 0)

    xt = {
        b: nc.alloc_sbuf_tensor(f"xt_{b}", [P, N_CHUNKS * DA], IN_DT)
        for b in bases
    }
    load_sem = {b: nc.alloc_semaphore(f"ld_{b}") for b in bases}
    # Per-PSUM-tensor completion sems: psA is final after the last pair's b0
    # matmul, psB after its b2 matmul — lets the psA copy start early.
    peA_sem = {b: nc.alloc_semaphore(f"peA_{b}") for b in bases}
    peB_sem = {b: nc.alloc_semaphore(f"peB_{b}") for b in bases}
    cpA_sem = {b: nc.alloc_semaphore(f"cpA_{b}") for b in bases}
    cpB_sem = {b: nc.alloc_semaphore(f"cpB_{b}") for b in bases}
    out_sem = {b: nc.alloc_semaphore(f"os_{b}") for b in bases}

    # Input loads (see module docstring for the pair layout).
    views = {}
    for gi, base in enumerate(bases):
        x_part = ins[base].rearrange("(p n) d -> p (n d)", p=P)
        views[base] = xt[base].ap().rearrange("p (h c d) -> p h c d", h=2, d=DA)
        a = 0
        for k in pair_groups[gi]:
            nc.sync.dma_start(
                out=views[base][:, :, a : a + k, :],
                in_=x_part[:, 2 * a * DA : 2 * (a + k) * DA],
            ).then_inc(load_sem[base], 16)
            a += k
        assert a == N_PAIRS

    # Matmuls: per pair, three block matmuls; each pair group's first
    # matmul is gated on its input DMA's completion sem.
    ps = {}
    for gi, base in enumerate(bases):
        v = views[base]
        psA = nc.alloc_psum_tensor(f"{base}_psA", [P, PS_W[0]], OUT_DT)
        psB = nc.alloc_psum_tensor(f"{base}_psB", [P, PS_W[1]], OUT_DT)
        ps[base] = (psA, psB)
        pe_sems = (peA_sem[base], peB_sem[base])
        p_ = 0
        for j, k in enumerate(pair_groups[gi]):
            nc.tensor.wait_ge(load_sem[base], 16 * (j + 1))
            for _ in range(k):
                first, last = p_ == 0, p_ == N_PAIRS - 1
                for rlo, rhi, clo, chi, pi, coff, st_, sp_, sem_i in MM_BLOCKS:
                    mm = nc.tensor.matmul(
                        ps[base][pi].ap()[0 : rhi - rlo, coff : coff + chi - clo],
                        v[:, :, p_, rlo:rhi],
                        v[:, :, p_, clo:chi],
                        start=(first and st_),
                        stop=(last and sp_),
                        perf_mode=mybir.MatmulPerfMode.DoubleRow,
                    )
                    if last and sem_i is not None:
                        # psA is final after the last pair's b0 matmul, psB
                        # after its b2 — per-tensor completion sems let the
                        # psA drain copy start early.
                        mm.then_inc(pe_sems[sem_i], 1)
                p_ += 1

    # PSUM -> fp16 staging drain: psA (322 cols) on Act, psB (260 cols) on
    # DVE, then a single store per tensor from SP once both copies land.
    for base in bases:
        psA, psB = ps[base]
        s = st[base].ap()
        nc.scalar.wait_ge(peA_sem[base], 1)
        nc.scalar.copy(s[:, 0 : PS_W[0]], psA.ap()[:, :]).then_inc(
            cpA_sem[base], 1
        )
        nc.vector.wait_ge(peB_sem[base], 1)
        nc.vector.tensor_copy(s[:, PS_W[0] : OUT_W], psB.ap()[:, :]).then_inc(
            cpB_sem[base], 1
        )
        # Store from SP: Act's sequencer wakes faster after the copies, but
        # its DGE delay is 784ns vs SP's 650ns, which nets out worse.
        nc.sync.wait_ge(cpA_sem[base], 1)
        nc.sync.wait_ge(cpB_sem[base], 1)
        nc.sync.dma_start(out=outs[base], in_=s[:, :]).then_inc(
            out_sem[base], 16
        )

    for base in bases:
        nc.sync.wait_ge(out_sem[base], 16)

    nc.compile()
    return nc


def _augment(x: np.ndarray) -> list[np.ndarray]:
    """Shard (B,T,D) over cores by batch, append the ones column, cast fp8."""
    x = np.asarray(x, dtype=np.float32)
    shards = []
    bpc = B // N_CORES
    for c in range(N_CORES):
        flat = x[c * bpc : (c + 1) * bpc].reshape(ROWS_PER_CORE, D)
        aug = np.empty((ROWS_PER_CORE, DA), dtype=np.float32)
        aug[:, :D] = flat
        aug[:, D] = 1.0
        # TRN fp8_e4m3 saturates at +-240 (vs OCP e4m3fn's 448); clip first
        # so out-of-range values can't hit the inf/nan encodings.
        shards.append(np.clip(aug, -240.0, 240.0).astype(IN_NP))
    return shards


def _assemble(packed: np.ndarray) -> np.ndarray:
    """Packed triangle blocks (128, 582) -> full symmetric (322, 322)."""
    s2a = np.zeros((DA, DA), dtype=np.float64)
    off = 0
    for rlo, rhi, clo, chi, _, _, _, _, _ in MM_BLOCKS:
        s2a[rlo:rhi, clo:chi] = packed[0 : rhi - rlo, off : off + chi - clo]
        off += chi - clo
    # mirror the strict upper block-triangle into the lower one
    s2a[128:256, 0:128] = s2a[0:128, 128:256].T
    s2a[256:DA, 0:128] = s2a[0:128, 256:DA].T
    s2a[256:DA, 128:256] = s2a[128:256, 256:DA].T
    return s2a


def _finalize(s2a_f: np.ndarray, s2a_r: np.ndarray) -> np.ndarray:
    def corr(s2a):
        n = s2a[D, D]
        s1 = s2a[:D, D]
        s2 = s2a[:D, :D]
        mu = s1 / n
        var = (np.diag(s2) - n * mu * mu) / (n - 1.0)
        sd = np.sqrt(var)
        return (s2 / n - np.outer(mu, mu)) / np.outer(sd, sd)

    gf = corr(s2a_f)
    gr = corr(s2a_r)
    i0, i1 = np.tril_indices(D)
    loss = np.abs(gf[i0, i1] - gr[i0, i1]).sum() / 10.0
    return np.array(loss, dtype=np.float32)


def kernel(x_fake: np.ndarray, x_real: np.ndarray, _trace=False):
    if "nc" not in _NC_CACHE:
        _NC_CACHE["nc"] = _build_program()
    nc = _NC_CACHE["nc"]

    fs = _augment(x_fake)
    rs = _augment(x_real)
    in_maps = [{"xf": fs[c], "xr": rs[c]} for c in range(N_CORES)]

    res = bass_utils.run_bass_kernel_spmd(
        nc, in_maps, core_ids=list(range(N_CORES)), trace=_trace
    )

    s2a_f = np.zeros((DA, DA), dtype=np.float64)
    s2a_r = np.zeros((DA, DA), dtype=np.float64)
    for c in range(N_CORES):
        s2a_f += _assemble(res.results[c]["gf"].astype(np.float64))
        s2a_r += _assemble(res.results[c]["gr"].astype(np.float64))

    loss = _finalize(s2a_f, s2a_r)
    if _trace:
        return loss, res
    return loss
